# revision 31
# baseline (speedup 1.0000x reference)
"""Trainium2 Bass kernel for nn_Attention_80779744903968.

Reference computation (B=32, T=512, S=1024, H=1024):
    z      = q @ W_in.T                  [B,T,H]
    scores = z @ enc_b.T                 [B,T,S]   (enc input is [S,B,H])
    p      = softmax(scores, axis=-1)    (the scores==0 -> -inf fill is a
                                          numerical no-op: row maxes are ~120,
                                          exp(0-max) == 0 in fp32)
    c      = p @ enc_b                   [B,T,H]
    out    = tanh([c, q] @ W_out.T + b)  [B,T,H]

Sharding: data-parallel over B across 8 cores (4 batches per core).
W_in / W_out replicated.

Precision strategy (PE matmuls):
  - z and scores need near-fp32 logits: the softmax is near-one-hot
    (logit std ~37) with near-tied rows. Both are computed as an fp16
    hi/lo split: x*y ~= xh*yh (fp16 main pass, fp32 PSUM accumulation)
    + (xh*yl + xl*yh) correction passes.
  - mode "f16x3": corrections in fp16 (absmax ~1.6e-3 vs reference).
  - mode "dr": corrections in fp8(e4m3) with perf_mode=DoubleRow at 2
    contraction-tiles per matmul (absmax ~2.5e-3); correction operands are
    pre-scaled by powers of 2 so both corr products share one PSUM scale.
  - downstream (p, enc, c, q, W_out) runs in plain fp16: p is in [0,1] and
    c/out magnitudes are O(1), so fp16's 2^-11 relative error is plenty.

All transposes (q -> [H,T], enc -> [H,S] per batch) are done on the host so
every device-side DMA is a contiguous natural-layout load; only the softmax
output p is transposed on-device (PE transpose-mode, fp16, 128x128 tiles).
"""
import os
import sys

import numpy as np

sys.path.insert(0, "/opt/trn_rl_repo")

import ml_dtypes  # noqa: E402

import concourse.bass as bass  # noqa: E402
import concourse.tile as tile  # noqa: E402
from concourse import bacc, mybir  # noqa: E402
from concourse.bass_utils import run_bass_kernel_spmd  # noqa: E402
from concourse.masks import make_identity  # noqa: E402

B, T, S, H = 32, 512, 1024, 1024
NCORES = 8
BL = B // NCORES  # batches per core
HT = H // 128     # h/i/k tiles per 1024
TT = T // 128     # t tiles
ST = S // 128     # s tiles
F16 = mybir.dt.float16
F32 = mybir.dt.float32
F8 = mybir.dt.float8e4
DR = mybir.MatmulPerfMode.DoubleRow

MODE = os.environ.get("KERNEL_MODE", "dr")

# power-of-2 scales for fp8 correction operands (products must share scale)
SC_WH, SC_WL, SC_QH, SC_QL = 2.0**4, 2.0**15, 1.0, 2.0**11   # z corr: 2^15
SC_ZH, SC_ZL, SC_EH, SC_EL = 1.0, 2.0**12, 1.0, 2.0**12      # s corr: 2^12

_CACHE = {}


def _build(mode, has_bias):
    nc = bacc.Bacc("TRN2", target_bir_lowering=False, debug=False,
                   num_devices=NCORES)

    def din(name, shape, dt=F16):
        return nc.dram_tensor(name, shape, dt, kind="ExternalInput").ap()

    qh_d = din("qh", [BL, H, T])
    eh_d = din("eh", [BL, H, S])
    en_d = din("en", [BL, S, H])
    wh_d = din("wh", [H, H])
    wo_d = din("wo", [2 * H, H])
    bias_d = din("bias", [128, H], F32)
    if mode == "dr":
        ql8_d = din("ql8", [BL, H, T], F8)
        el8_d = din("el8", [BL, H, S], F8)
        qh8_d = din("qh8", [BL, H, T], F8)
        eh8_d = din("eh8", [BL, H, S], F8)
        wh8_d = din("wh8", [H, H], F8)
        wl8_d = din("wl8", [H, H], F8)
    else:
        wl_d = din("wl", [H, H])
        ql_d = din("ql", [BL, H, T])
        el_d = din("el", [BL, H, S])
    out_d = nc.dram_tensor("out", [BL, T, H], F32, kind="ExternalOutput").ap()

    with tile.TileContext(nc) as tc:
        with (
            tc.tile_pool(name="weights", bufs=1) as wp,
            tc.tile_pool(name="qin", bufs=2) as qp,
            tc.tile_pool(name="ein", bufs=1) as ep,
            tc.tile_pool(name="enin", bufs=1) as enp,
            tc.tile_pool(name="zbuf", bufs=1) as zp,
            tc.tile_pool(name="scratch", bufs=2) as scrp,
            tc.tile_pool(name="scores", bufs=2) as scp,
            tc.tile_pool(name="pbuf", bufs=3) as pp,
            tc.tile_pool(name="ptbuf", bufs=1) as ptp,
            tc.tile_pool(name="ctbuf", bufs=1) as ctp,
            tc.tile_pool(name="ostage", bufs=3) as op,
            tc.tile_pool(name="stats", bufs=8) as stp,
            tc.tile_pool(name="psmm", bufs=4, space="PSUM") as psmm,
            tc.tile_pool(name="psc", bufs=2, space="PSUM") as psc,
            tc.tile_pool(name="pstr", bufs=2, space="PSUM") as pstr,
        ):
            # --- resident weights / constants ---
            # queue order matters at startup: wh then b0's qh go first so the
            # PE can start the z main pass after only 3MB of DMA.
            # split the startup-critical loads across both HWDGE rings
            wh_t = wp.tile([128, HT, H], F16)
            wh_r = wh_d.rearrange("(ht p) i -> p ht i", p=128)
            nc.sync.dma_start(wh_t[:, 0:HT // 2, :], wh_r[:, 0:HT // 2, :])
            nc.scalar.dma_start(wh_t[:, HT // 2:, :], wh_r[:, HT // 2:, :])
            qh_first = qp.tile([128, HT, T], F16, tag="qh")
            qh_r = qh_d[0].rearrange("(ht p) t -> p ht t", p=128)
            nc.sync.dma_start(qh_first[:, 0:HT // 2, :], qh_r[:, 0:HT // 2, :])
            nc.scalar.dma_start(qh_first[:, HT // 2:, :], qh_r[:, HT // 2:, :])
            if mode == "dr":
                wl8_t = wp.tile([128, HT, H], F8)
                nc.scalar.dma_start(
                    wl8_t[:], wl8_d.rearrange("(ht p) i -> p ht i", p=128))
                wh8_t = wp.tile([128, HT, H], F8)
                nc.scalar.dma_start(
                    wh8_t[:], wh8_d.rearrange("(ht p) i -> p ht i", p=128))
                qh8_first = qp.tile([128, HT, T], F8, tag="qh8", bufs=1)
                nc.sync.dma_start(
                    qh8_first[:], qh8_d[0].rearrange("(ht p) t -> p ht t", p=128))
                ql8_first = qp.tile([128, HT, T], F8, tag="ql8", bufs=1)
                nc.sync.dma_start(
                    ql8_first[:], ql8_d[0].rearrange("(ht p) t -> p ht t", p=128))
            else:
                wl_t = wp.tile([128, HT, H], F16)
                nc.sync.dma_start(
                    wl_t[:], wl_d.rearrange("(ht p) i -> p ht i", p=128))
            ident = wp.tile([128, 128], F16)
            make_identity(nc, ident[:])
            wo_t = None
            bias_t = None

            for b in range(BL):
                if b == 0:
                    qh_t = qh_first
                else:
                    qh_t = qp.tile([128, HT, T], F16, tag="qh")
                    nc.sync.dma_start(
                        qh_t[:], qh_d[b].rearrange("(ht p) t -> p ht t", p=128))
                if mode == "dr":
                    if b == 0:
                        qh8_t, ql8_t = qh8_first, ql8_first
                    else:
                        qh8_t = qp.tile([128, HT, T], F8, tag="qh8", bufs=1)
                        nc.sync.dma_start(
                            qh8_t[:], qh8_d[b].rearrange("(ht p) t -> p ht t", p=128))
                        ql8_t = qp.tile([128, HT, T], F8, tag="ql8", bufs=1)
                        nc.sync.dma_start(
                            ql8_t[:], ql8_d[b].rearrange("(ht p) t -> p ht t", p=128))
                else:
                    ql_t = qp.tile([128, HT, T], F16, tag="ql")
                    nc.sync.dma_start(
                        ql_t[:], ql_d[b].rearrange("(ht p) t -> p ht t", p=128))
                eh_t = ep.tile([128, HT, S], F16, tag="eh")
                nc.sync.dma_start(
                    eh_t[:], eh_d[b].rearrange("(it p) s -> p it s", p=128))
                if mode == "dr":
                    eh8_t = ep.tile([128, HT, S], F8, tag="eh8")
                    nc.sync.dma_start(
                        eh8_t[:], eh8_d[b].rearrange("(it p) s -> p it s", p=128))
                    el8_t = ep.tile([128, HT, S], F8, tag="el8")
                    nc.sync.dma_start(
                        el8_t[:], el8_d[b].rearrange("(it p) s -> p it s", p=128))
                else:
                    el_t = ep.tile([128, HT, S], F16, tag="el")
                    nc.sync.dma_start(
                        el_t[:], el_d[b].rearrange("(it p) s -> p it s", p=128))
                if wo_t is None:
                    # deferred: b0's scores inputs take queue priority
                    wo_t = wp.tile([128, 2 * HT, H], F16)
                    nc.sync.dma_start(
                        wo_t[:], wo_d.rearrange("(kt p) h -> p kt h", p=128))
                    bias_t = wp.tile([128, H], F32)
                    nc.sync.dma_start(bias_t[:], bias_d)
                en_t = enp.tile([128, ST, H], F16, tag="en")
                nc.sync.dma_start(
                    en_t[:], en_d[b].rearrange("(st p) k -> p st k", p=128))

                # --- zT = W_inT.T @ qT (hi/lo) -> zh (f16) + lo forms ---
                zh_t = zp.tile([128, HT, T], F16, tag="zh")
                if mode == "dr":
                    zh8_t = zp.tile([128, HT, T], F8, tag="zh8")
                    zl8_t = zp.tile([128, HT, T], F8, tag="zl8")
                else:
                    zl_t = zp.tile([128, HT, T], F16, tag="zl")

                def z_evict(it, zps, zcorr=None):
                    if mode == "dr":
                        corr_s = scrp.tile([128, T], F32, tag="corrs")
                        nc.vector.tensor_scalar_mul(
                            corr_s[:], zcorr[:], 1.0 / (SC_WH * SC_QL))
                        comb = scrp.tile([128, T], F32, tag="comb")
                        nc.vector.tensor_add(comb[:], zps[:], corr_s[:])
                        nc.vector.tensor_copy(zh_t[:, it, :], comb[:])
                        zl_tmp = scrp.tile([128, T], F16, tag="zltmp")
                        nc.vector.tensor_sub(zl_tmp[:], comb[:], zh_t[:, it, :])
                        nc.vector.tensor_scalar_mul(
                            zl8_t[:, it, :], zl_tmp[:], SC_ZL)
                        nc.vector.tensor_copy(zh8_t[:, it, :], zh_t[:, it, :])
                    else:
                        nc.vector.tensor_copy(zh_t[:, it, :], zps[:])
                        nc.vector.tensor_sub(zl_t[:, it, :], zps[:],
                                             zh_t[:, it, :])

                def z_corr_mms(zcorr, it, first, last):
                    # fp8 DoubleRow corrections: (wl8*qh8 + wh8*ql8) * 2^-15
                    j = 0
                    n = HT  # 2 passes x HT/2 pair-matmuls
                    for lhs, rhs in ((wl8_t, qh8_t), (wh8_t, ql8_t)):
                        for k in range(HT // 2):
                            nc.tensor.matmul(
                                zcorr[:],
                                lhs[:, 2 * k:2 * k + 2, it * 128:(it + 1) * 128],
                                rhs[:, 2 * k:2 * k + 2, :],
                                start=(first and j == 0),
                                stop=(last and j == n - 1),
                                perf_mode=DR)
                            j += 1

                if b == 0 and mode != "dr":
                    # pass-major over quads of i-tiles: the first 32 matmuls
                    # need only wh+qh (3MB of DMA), so the PE starts early.
                    for quad in range(HT // 4):
                        its = range(quad * 4, quad * 4 + 4)
                        zpss = {it: psmm.tile([128, T], F32, tag="mm", name=f"zps{it}")
                                for it in its}
                        if mode == "dr":
                            zcorrs = {it: psc.tile([128, T], F32, tag="mmc", name=f"zcorr{it}")
                                      for it in its}
                            for it in its:
                                for ht in range(HT):
                                    nc.tensor.matmul(
                                        zpss[it][:],
                                        wh_t[:, ht, it * 128:(it + 1) * 128],
                                        qh_t[:, ht, :],
                                        start=(ht == 0), stop=(ht == HT - 1))
                            for it in its:
                                z_corr_mms(zcorrs[it], it, True, True)
                            for it in its:
                                z_evict(it, zpss[it], zcorrs[it])
                        else:
                            passes = ((wh_t, qh_t), (wl_t, qh_t), (wh_t, ql_t))
                            for pi, (lhs, rhs) in enumerate(passes):
                                for it in its:
                                    for ht in range(HT):
                                        nc.tensor.matmul(
                                            zpss[it][:],
                                            lhs[:, ht, it * 128:(it + 1) * 128],
                                            rhs[:, ht, :],
                                            start=(pi == 0 and ht == 0),
                                            stop=(pi == 2 and ht == HT - 1))
                            for it in its:
                                z_evict(it, zpss[it])
                else:
                    for it in range(HT):
                        zps = psmm.tile([128, T], F32, tag="mm")
                        if mode == "dr":
                            for ht in range(HT):
                                nc.tensor.matmul(
                                    zps[:],
                                    wh_t[:, ht, it * 128:(it + 1) * 128],
                                    qh_t[:, ht, :],
                                    start=(ht == 0), stop=(ht == HT - 1))
                            zcorr = psc.tile([128, T], F32, tag="mmc")
                            z_corr_mms(zcorr, it, True, True)
                            z_evict(it, zps, zcorr)
                        else:
                            j = 0
                            for lhs, rhs in ((wh_t, qh_t), (wh_t, ql_t),
                                             (wl_t, qh_t)):
                                for ht in range(HT):
                                    nc.tensor.matmul(
                                        zps[:],
                                        lhs[:, ht, it * 128:(it + 1) * 128],
                                        rhs[:, ht, :],
                                        start=(j == 0), stop=(j == 3 * HT - 1))
                                    j += 1
                            z_evict(it, zps)

                # --- scores + softmax -> p (f16, normalized) ---
                p_tiles = []
                for tt in range(TT):
                    sc_t = scp.tile([128, S], F32, tag="sc")
                    for sc in range(2):
                        sps = psmm.tile([128, 512], F32, tag="mm")
                        for it in range(HT):
                            nc.tensor.matmul(
                                sps[:],
                                zh_t[:, it, tt * 128:(tt + 1) * 128],
                                eh_t[:, it, sc * 512:(sc + 1) * 512],
                                start=(it == 0),
                                stop=(mode == "dr" and it == HT - 1))
                        if mode == "dr":
                            scorr = psc.tile([128, 512], F32, tag="mmc")
                            j = 0
                            for lhs, rhs in ((zl8_t, eh8_t), (zh8_t, el8_t)):
                                for k in range(HT // 2):
                                    nc.tensor.matmul(
                                        scorr[:],
                                        lhs[:, 2 * k:2 * k + 2,
                                            tt * 128:(tt + 1) * 128],
                                        rhs[:, 2 * k:2 * k + 2,
                                            sc * 512:(sc + 1) * 512],
                                        start=(j == 0), stop=(j == HT - 1),
                                        perf_mode=DR)
                                    j += 1
                            corr_s = scrp.tile([128, 512], F32,
                                                tag="corrs", name="scorr_s")
                            nc.vector.tensor_scalar_mul(
                                corr_s[:], scorr[:], 1.0 / (SC_ZL * SC_EH))
                            nc.vector.tensor_add(
                                sc_t[:, sc * 512:(sc + 1) * 512],
                                sps[:], corr_s[:])
                        else:
                            j = 0
                            for lhs, rhs in ((zh_t, el_t), (zl_t, eh_t)):
                                for it in range(HT):
                                    nc.tensor.matmul(
                                        sps[:],
                                        lhs[:, it, tt * 128:(tt + 1) * 128],
                                        rhs[:, it, sc * 512:(sc + 1) * 512],
                                        start=False, stop=(j == 2 * HT - 1))
                                    j += 1
                            nc.vector.tensor_copy(
                                sc_t[:, sc * 512:(sc + 1) * 512], sps[:])
                    # softmax over free dim (s)
                    negmax = stp.tile([128, 1], F32, tag="nm")
                    nc.vector.reduce_max(out=negmax[:], in_=sc_t[:],
                                         axis=mybir.AxisListType.X, negate=True)
                    p_t = pp.tile([128, S], F16, tag="p")
                    nc.scalar.activation(
                        out=p_t[:], in_=sc_t[:],
                        func=mybir.ActivationFunctionType.Exp,
                        bias=negmax[:], scale=1.0)
                    ssum = stp.tile([128, 1], F32, tag="ss")
                    nc.vector.reduce_sum(out=ssum[:], in_=p_t[:],
                                         axis=mybir.AxisListType.X)
                    rsum = stp.tile([128, 1], F32, tag="rs")
                    nc.vector.reciprocal(rsum[:], ssum[:])
                    nc.vector.tensor_scalar_mul(p_t[:], p_t[:], rsum[:])
                    p_tiles.append(p_t)

                # --- transpose p -> pT [s, t] (PE transpose 128x128) ---
                pt_t = ptp.tile([128, ST, T], F16, tag="pt")
                for tt in range(TT):
                    for st in range(ST):
                        tps = pstr.tile([128, 128], F16, tag="tr")
                        nc.tensor.transpose(
                            tps[:], p_tiles[tt][:, st * 128:(st + 1) * 128],
                            ident[:])
                        nc.vector.tensor_copy(
                            pt_t[:, st, tt * 128:(tt + 1) * 128], tps[:])

                # --- cT = enc_nat.T @ pT -> [k, t] f16 ---
                ct_t = ctp.tile([128, HT, T], F16, tag="ct")
                for kt in range(HT):
                    cps = psmm.tile([128, T], F32, tag="mm")
                    for st in range(ST):
                        nc.tensor.matmul(
                            cps[:],
                            en_t[:, st, kt * 128:(kt + 1) * 128],
                            pt_t[:, st, :],
                            start=(st == 0), stop=(st == ST - 1))
                    nc.vector.tensor_copy(ct_t[:, kt, :], cps[:])

                # --- out = tanh(cT.T @ WcT + qT.T @ WqT + b) ---
                for tt in range(TT):
                    for hc in range(2):
                        ops = psmm.tile([128, 512], F32, tag="mm")
                        # q-part first: gives tail cT evictions extra slack
                        for ht in range(HT):
                            nc.tensor.matmul(
                                ops[:],
                                qh_t[:, ht, tt * 128:(tt + 1) * 128],
                                wo_t[:, HT + ht, hc * 512:(hc + 1) * 512],
                                start=(ht == 0), stop=False)
                        for kt in range(HT):
                            nc.tensor.matmul(
                                ops[:],
                                ct_t[:, kt, tt * 128:(tt + 1) * 128],
                                wo_t[:, kt, hc * 512:(hc + 1) * 512],
                                start=False, stop=(kt == HT - 1))
                        ost = op.tile([128, 512], F32, tag="os")
                        nc.vector.tensor_add(
                            ost[:], ops[:], bias_t[:, hc * 512:(hc + 1) * 512])
                        nc.scalar.activation(
                            out=ost[:], in_=ost[:],
                            func=mybir.ActivationFunctionType.Tanh)
                        nc.sync.dma_start(
                            out_d[b, tt * 128:(tt + 1) * 128,
                                  hc * 512:(hc + 1) * 512],
                            ost[:])

    nc.compile()
    return nc


def _get_nc(has_bias):
    key = ("nc", has_bias)
    if key not in _CACHE:
        _CACHE[key] = _build(MODE, has_bias)
    return _CACHE[key]


def _split16(x):
    hi = x.astype(np.float16)
    lo = (x - hi.astype(np.float32)).astype(np.float32)
    return hi, lo


def _f8(x, scale):
    return (np.asarray(x, np.float32) * np.float32(scale)).astype(
        ml_dtypes.float8_e4m3)


def kernel(query, encoder_outputs, src_lengths, W_in, W_out, b_out):
    query = np.asarray(query, np.float32)
    enc = np.asarray(encoder_outputs, np.float32)
    W_in = np.asarray(W_in, np.float32)
    W_out = np.asarray(W_out, np.float32)
    b_out = np.asarray(b_out, np.float32)

    # host-side layout prep (transposes + fp16 hi/lo splits)
    qT = np.ascontiguousarray(query.transpose(0, 2, 1))        # [B, H, T]
    qh, ql = _split16(qT)
    encT = np.ascontiguousarray(enc.transpose(1, 2, 0))        # [B, H, S]
    eh, el = _split16(encT)
    en = np.ascontiguousarray(enc.transpose(1, 0, 2)).astype(np.float16)
    whf, wlf = _split16(np.ascontiguousarray(W_in.T))          # [H(h), H(i)]
    wo = np.ascontiguousarray(W_out.T).astype(np.float16)      # [2H, H]
    bias = np.ascontiguousarray(
        np.broadcast_to(b_out[None, :], (128, H)), np.float32)

    common = {"wh": whf, "wo": wo, "bias": bias}
    if MODE == "dr":
        common["wh8"] = _f8(whf.astype(np.float32), SC_WH)
        common["wl8"] = _f8(wlf, SC_WL)
    else:
        common["wl"] = wlf.astype(np.float16)

    in_maps = []
    for c in range(NCORES):
        sl = slice(c * BL, (c + 1) * BL)
        m = {
            "qh": np.ascontiguousarray(qh[sl]),
            "eh": np.ascontiguousarray(eh[sl]),
            "en": np.ascontiguousarray(en[sl]),
            **common,
        }
        if MODE == "dr":
            m["qh8"] = _f8(qh[sl].astype(np.float32), SC_QH)
            m["ql8"] = _f8(ql[sl], SC_QL)
            m["eh8"] = _f8(eh[sl].astype(np.float32), SC_EH)
            m["el8"] = _f8(el[sl], SC_EL)
        else:
            m["ql"] = np.ascontiguousarray(ql[sl]).astype(np.float16)
            m["el"] = np.ascontiguousarray(el[sl]).astype(np.float16)
        in_maps.append(m)

    nc = _get_nc(bool(np.any(b_out)))
    trace = bool(int(os.environ.get("KERNEL_TRACE", "0")))
    res = run_bass_kernel_spmd(nc, in_maps, core_ids=list(range(NCORES)),
                               trace=trace)
    if trace:
        _CACHE["last_exec_time_ns"] = res.exec_time_ns
        _CACHE["last_results"] = res
    out = np.concatenate([r["out"] for r in res.results], axis=0)
    return out


# revision 33
# speedup vs baseline: 1.0082x; 1.0082x over previous
"""Trainium2 Bass kernel for nn_Attention_80779744903968.

Reference computation (B=32, T=512, S=1024, H=1024):
    z      = q @ W_in.T                  [B,T,H]
    scores = z @ enc_b.T                 [B,T,S]   (enc input is [S,B,H])
    p      = softmax(scores, axis=-1)    (the scores==0 -> -inf fill is a
                                          numerical no-op: row maxes are ~120,
                                          exp(0-max) == 0 in fp32)
    c      = p @ enc_b                   [B,T,H]
    out    = tanh([c, q] @ W_out.T + b)  [B,T,H]

Sharding: data-parallel over B across 8 cores (4 batches per core).
W_in / W_out replicated.

Precision strategy (PE matmuls):
  - z and scores need near-fp32 logits: the softmax is near-one-hot
    (logit std ~37) with near-tied rows. Both are computed as an fp16
    hi/lo split: x*y ~= xh*yh (fp16 main pass, fp32 PSUM accumulation)
    + (xh*yl + xl*yh) correction passes.
  - mode "f16x3": corrections in fp16 (absmax ~1.6e-3 vs reference).
  - mode "dr": corrections in fp8(e4m3) with perf_mode=DoubleRow at 2
    contraction-tiles per matmul (absmax ~2.5e-3); correction operands are
    pre-scaled by powers of 2 so both corr products share one PSUM scale.
  - downstream (p, enc, c, q, W_out) runs in plain fp16: p is in [0,1] and
    c/out magnitudes are O(1), so fp16's 2^-11 relative error is plenty.

All transposes (q -> [H,T], enc -> [H,S] per batch) are done on the host so
every device-side DMA is a contiguous natural-layout load; only the softmax
output p is transposed on-device (PE transpose-mode, fp16, 128x128 tiles).
"""
import os
import sys

import numpy as np

sys.path.insert(0, "/opt/trn_rl_repo")

import ml_dtypes  # noqa: E402

import concourse.bass as bass  # noqa: E402
import concourse.tile as tile  # noqa: E402
from concourse import bacc, mybir  # noqa: E402
from concourse.bass_utils import run_bass_kernel_spmd  # noqa: E402
from concourse.masks import make_identity  # noqa: E402

B, T, S, H = 32, 512, 1024, 1024
NCORES = 8
BL = B // NCORES  # batches per core
HT = H // 128     # h/i/k tiles per 1024
TT = T // 128     # t tiles
ST = S // 128     # s tiles
F16 = mybir.dt.float16
F32 = mybir.dt.float32
F8 = mybir.dt.float8e4
DR = mybir.MatmulPerfMode.DoubleRow

MODE = os.environ.get("KERNEL_MODE", "dr")

# power-of-2 scales for fp8 correction operands (products must share scale)
SC_WH, SC_WL, SC_QH, SC_QL = 2.0**4, 2.0**15, 1.0, 2.0**11   # z corr: 2^15
SC_ZH, SC_ZL, SC_EH, SC_EL = 1.0, 2.0**12, 1.0, 2.0**12      # s corr: 2^12

_CACHE = {}


def _build(mode, has_bias):
    nc = bacc.Bacc("TRN2", target_bir_lowering=False, debug=False,
                   num_devices=NCORES)

    def din(name, shape, dt=F16):
        return nc.dram_tensor(name, shape, dt, kind="ExternalInput").ap()

    qh_d = din("qh", [BL, H, T])
    eh_d = din("eh", [BL, H, S])
    en_d = din("en", [BL, S, H])
    wh_d = din("wh", [H, H])
    wo_d = din("wo", [2 * H, H])
    bias_d = din("bias", [128, H], F32)
    if mode == "dr":
        ql8_d = din("ql8", [BL, H, T], F8)
        el8_d = din("el8", [BL, H, S], F8)
        qh8_d = din("qh8", [BL, H, T], F8)
        eh8_d = din("eh8", [BL, H, S], F8)
        wh8_d = din("wh8", [H, H], F8)
        wl8_d = din("wl8", [H, H], F8)
    else:
        wl_d = din("wl", [H, H])
        ql_d = din("ql", [BL, H, T])
        el_d = din("el", [BL, H, S])
    out_d = nc.dram_tensor("out", [BL, T, H], F32, kind="ExternalOutput").ap()

    with tile.TileContext(nc) as tc:
        with (
            tc.tile_pool(name="weights", bufs=1) as wp,
            tc.tile_pool(name="qin", bufs=2) as qp,
            tc.tile_pool(name="ein", bufs=1) as ep,
            tc.tile_pool(name="enin", bufs=1) as enp,
            tc.tile_pool(name="zbuf", bufs=1) as zp,
            tc.tile_pool(name="scratch", bufs=2) as scrp,
            tc.tile_pool(name="scores", bufs=2) as scp,
            tc.tile_pool(name="pbuf", bufs=3) as pp,
            tc.tile_pool(name="ptbuf", bufs=1) as ptp,
            tc.tile_pool(name="ctbuf", bufs=1) as ctp,
            tc.tile_pool(name="ostage", bufs=3) as op,
            tc.tile_pool(name="stats", bufs=8) as stp,
            tc.tile_pool(name="psmm", bufs=4, space="PSUM") as psmm,
            tc.tile_pool(name="psc", bufs=2, space="PSUM") as psc,
            tc.tile_pool(name="pstr", bufs=2, space="PSUM") as pstr,
        ):
            # --- resident weights / constants ---
            # queue order matters at startup: wh then b0's qh go first so the
            # PE can start the z main pass after only 3MB of DMA.
            wh_t = wp.tile([128, HT, H], F16)
            nc.sync.dma_start(wh_t[:], wh_d.rearrange("(ht p) i -> p ht i", p=128))
            qh_first = qp.tile([128, HT, T], F16, tag="qh")
            nc.sync.dma_start(
                qh_first[:], qh_d[0].rearrange("(ht p) t -> p ht t", p=128))
            if mode == "dr":
                wl8_t = wp.tile([128, HT, H], F8)
                nc.sync.dma_start(
                    wl8_t[:], wl8_d.rearrange("(ht p) i -> p ht i", p=128))
                wh8_t = wp.tile([128, HT, H], F8)
                nc.sync.dma_start(
                    wh8_t[:], wh8_d.rearrange("(ht p) i -> p ht i", p=128))
                qh8_first = qp.tile([128, HT, T], F8, tag="qh8", bufs=1)
                nc.sync.dma_start(
                    qh8_first[:], qh8_d[0].rearrange("(ht p) t -> p ht t", p=128))
                ql8_first = qp.tile([128, HT, T], F8, tag="ql8", bufs=1)
                nc.sync.dma_start(
                    ql8_first[:], ql8_d[0].rearrange("(ht p) t -> p ht t", p=128))
            else:
                wl_t = wp.tile([128, HT, H], F16)
                nc.sync.dma_start(
                    wl_t[:], wl_d.rearrange("(ht p) i -> p ht i", p=128))
            ident = wp.tile([128, 128], F16)
            make_identity(nc, ident[:])
            wo_t = None
            bias_t = None

            for b in range(BL):
                if b == 0:
                    qh_t = qh_first
                else:
                    qh_t = qp.tile([128, HT, T], F16, tag="qh")
                    nc.sync.dma_start(
                        qh_t[:], qh_d[b].rearrange("(ht p) t -> p ht t", p=128))
                if mode == "dr":
                    if b == 0:
                        qh8_t, ql8_t = qh8_first, ql8_first
                    else:
                        qh8_t = qp.tile([128, HT, T], F8, tag="qh8", bufs=1)
                        nc.sync.dma_start(
                            qh8_t[:], qh8_d[b].rearrange("(ht p) t -> p ht t", p=128))
                        ql8_t = qp.tile([128, HT, T], F8, tag="ql8", bufs=1)
                        nc.sync.dma_start(
                            ql8_t[:], ql8_d[b].rearrange("(ht p) t -> p ht t", p=128))
                else:
                    ql_t = qp.tile([128, HT, T], F16, tag="ql")
                    nc.sync.dma_start(
                        ql_t[:], ql_d[b].rearrange("(ht p) t -> p ht t", p=128))
                eh_t = ep.tile([128, HT, S], F16, tag="eh")
                nc.sync.dma_start(
                    eh_t[:], eh_d[b].rearrange("(it p) s -> p it s", p=128))
                if mode == "dr":
                    eh8_t = ep.tile([128, HT, S], F8, tag="eh8")
                    nc.sync.dma_start(
                        eh8_t[:], eh8_d[b].rearrange("(it p) s -> p it s", p=128))
                    el8_t = ep.tile([128, HT, S], F8, tag="el8")
                    nc.sync.dma_start(
                        el8_t[:], el8_d[b].rearrange("(it p) s -> p it s", p=128))
                else:
                    el_t = ep.tile([128, HT, S], F16, tag="el")
                    nc.sync.dma_start(
                        el_t[:], el_d[b].rearrange("(it p) s -> p it s", p=128))
                if wo_t is None:
                    # deferred: b0's scores inputs take queue priority
                    wo_t = wp.tile([128, 2 * HT, H], F16)
                    nc.sync.dma_start(
                        wo_t[:], wo_d.rearrange("(kt p) h -> p kt h", p=128))
                    bias_t = wp.tile([128, H], F32)
                    nc.sync.dma_start(bias_t[:], bias_d)
                en_t = enp.tile([128, ST, H], F16, tag="en")
                nc.sync.dma_start(
                    en_t[:], en_d[b].rearrange("(st p) k -> p st k", p=128))

                # --- zT = W_inT.T @ qT (hi/lo) -> zh (f16) + lo forms ---
                zh_t = zp.tile([128, HT, T], F16, tag="zh")
                if mode == "dr":
                    zh8_t = zp.tile([128, HT, T], F8, tag="zh8")
                    zl8_t = zp.tile([128, HT, T], F8, tag="zl8")
                else:
                    zl_t = zp.tile([128, HT, T], F16, tag="zl")

                def z_evict(it, zps, zcorr=None):
                    if mode == "dr":
                        comb = scrp.tile([128, T], F32, tag="comb")
                        nc.vector.tensor_copy(comb[:], zps[:])
                        nc.vector.scalar_tensor_tensor(
                            out=comb[:], in0=zcorr[:],
                            scalar=1.0 / (SC_WH * SC_QL), in1=comb[:],
                            op0=mybir.AluOpType.mult, op1=mybir.AluOpType.add)
                        nc.vector.tensor_copy(zh_t[:, it, :], comb[:])
                        zl_tmp = scrp.tile([128, T], F16, tag="zltmp")
                        nc.vector.tensor_sub(zl_tmp[:], comb[:], zh_t[:, it, :])
                        nc.vector.tensor_scalar_mul(
                            zl8_t[:, it, :], zl_tmp[:], SC_ZL)
                        nc.vector.tensor_copy(zh8_t[:, it, :], zh_t[:, it, :])
                    else:
                        nc.vector.tensor_copy(zh_t[:, it, :], zps[:])
                        nc.vector.tensor_sub(zl_t[:, it, :], zps[:],
                                             zh_t[:, it, :])

                def z_corr_mms(zcorr, it, first, last):
                    # fp8 DoubleRow corrections: (wl8*qh8 + wh8*ql8) * 2^-15
                    j = 0
                    n = HT  # 2 passes x HT/2 pair-matmuls
                    for lhs, rhs in ((wl8_t, qh8_t), (wh8_t, ql8_t)):
                        for k in range(HT // 2):
                            nc.tensor.matmul(
                                zcorr[:],
                                lhs[:, 2 * k:2 * k + 2, it * 128:(it + 1) * 128],
                                rhs[:, 2 * k:2 * k + 2, :],
                                start=(first and j == 0),
                                stop=(last and j == n - 1),
                                perf_mode=DR)
                            j += 1

                if b == 0 and mode != "dr":
                    # pass-major over quads of i-tiles: the first 32 matmuls
                    # need only wh+qh (3MB of DMA), so the PE starts early.
                    for quad in range(HT // 4):
                        its = range(quad * 4, quad * 4 + 4)
                        zpss = {it: psmm.tile([128, T], F32, tag="mm", name=f"zps{it}")
                                for it in its}
                        if mode == "dr":
                            zcorrs = {it: psc.tile([128, T], F32, tag="mmc", name=f"zcorr{it}")
                                      for it in its}
                            for it in its:
                                for ht in range(HT):
                                    nc.tensor.matmul(
                                        zpss[it][:],
                                        wh_t[:, ht, it * 128:(it + 1) * 128],
                                        qh_t[:, ht, :],
                                        start=(ht == 0), stop=(ht == HT - 1))
                            for it in its:
                                z_corr_mms(zcorrs[it], it, True, True)
                            for it in its:
                                z_evict(it, zpss[it], zcorrs[it])
                        else:
                            passes = ((wh_t, qh_t), (wl_t, qh_t), (wh_t, ql_t))
                            for pi, (lhs, rhs) in enumerate(passes):
                                for it in its:
                                    for ht in range(HT):
                                        nc.tensor.matmul(
                                            zpss[it][:],
                                            lhs[:, ht, it * 128:(it + 1) * 128],
                                            rhs[:, ht, :],
                                            start=(pi == 0 and ht == 0),
                                            stop=(pi == 2 and ht == HT - 1))
                            for it in its:
                                z_evict(it, zpss[it])
                else:
                    for it in range(HT):
                        zps = psmm.tile([128, T], F32, tag="mm")
                        if mode == "dr":
                            for ht in range(HT):
                                nc.tensor.matmul(
                                    zps[:],
                                    wh_t[:, ht, it * 128:(it + 1) * 128],
                                    qh_t[:, ht, :],
                                    start=(ht == 0), stop=(ht == HT - 1))
                            zcorr = psc.tile([128, T], F32, tag="mmc")
                            z_corr_mms(zcorr, it, True, True)
                            z_evict(it, zps, zcorr)
                        else:
                            j = 0
                            for lhs, rhs in ((wh_t, qh_t), (wh_t, ql_t),
                                             (wl_t, qh_t)):
                                for ht in range(HT):
                                    nc.tensor.matmul(
                                        zps[:],
                                        lhs[:, ht, it * 128:(it + 1) * 128],
                                        rhs[:, ht, :],
                                        start=(j == 0), stop=(j == 3 * HT - 1))
                                    j += 1
                            z_evict(it, zps)

                # --- scores + softmax -> p (f16, normalized) ---
                p_tiles = []
                for tt in range(TT):
                    sc_t = scp.tile([128, S], F32, tag="sc")
                    for sc in range(2):
                        sps = psmm.tile([128, 512], F32, tag="mm")
                        for it in range(HT):
                            nc.tensor.matmul(
                                sps[:],
                                zh_t[:, it, tt * 128:(tt + 1) * 128],
                                eh_t[:, it, sc * 512:(sc + 1) * 512],
                                start=(it == 0),
                                stop=(mode == "dr" and it == HT - 1))
                        if mode == "dr":
                            scorr = psc.tile([128, 512], F32, tag="mmc")
                            j = 0
                            for lhs, rhs in ((zl8_t, eh8_t), (zh8_t, el8_t)):
                                for k in range(HT // 2):
                                    nc.tensor.matmul(
                                        scorr[:],
                                        lhs[:, 2 * k:2 * k + 2,
                                            tt * 128:(tt + 1) * 128],
                                        rhs[:, 2 * k:2 * k + 2,
                                            sc * 512:(sc + 1) * 512],
                                        start=(j == 0), stop=(j == HT - 1),
                                        perf_mode=DR)
                                    j += 1
                            chunk = sc_t[:, sc * 512:(sc + 1) * 512]
                            nc.vector.tensor_copy(chunk, sps[:])
                            nc.vector.scalar_tensor_tensor(
                                out=chunk, in0=scorr[:],
                                scalar=1.0 / (SC_ZL * SC_EH), in1=chunk,
                                op0=mybir.AluOpType.mult,
                                op1=mybir.AluOpType.add)
                        else:
                            j = 0
                            for lhs, rhs in ((zh_t, el_t), (zl_t, eh_t)):
                                for it in range(HT):
                                    nc.tensor.matmul(
                                        sps[:],
                                        lhs[:, it, tt * 128:(tt + 1) * 128],
                                        rhs[:, it, sc * 512:(sc + 1) * 512],
                                        start=False, stop=(j == 2 * HT - 1))
                                    j += 1
                            nc.vector.tensor_copy(
                                sc_t[:, sc * 512:(sc + 1) * 512], sps[:])
                    # softmax over free dim (s)
                    negmax = stp.tile([128, 1], F32, tag="nm")
                    nc.vector.reduce_max(out=negmax[:], in_=sc_t[:],
                                         axis=mybir.AxisListType.X, negate=True)
                    p_t = pp.tile([128, S], F16, tag="p")
                    nc.scalar.activation(
                        out=p_t[:], in_=sc_t[:],
                        func=mybir.ActivationFunctionType.Exp,
                        bias=negmax[:], scale=1.0)
                    ssum = stp.tile([128, 1], F32, tag="ss")
                    nc.vector.reduce_sum(out=ssum[:], in_=p_t[:],
                                         axis=mybir.AxisListType.X)
                    rsum = stp.tile([128, 1], F32, tag="rs")
                    nc.vector.reciprocal(rsum[:], ssum[:])
                    nc.vector.tensor_scalar_mul(p_t[:], p_t[:], rsum[:])
                    p_tiles.append(p_t)

                # --- transpose p -> pT [s, t] (PE transpose 128x128) ---
                pt_t = ptp.tile([128, ST, T], F16, tag="pt")
                for tt in range(TT):
                    for st in range(ST):
                        tps = pstr.tile([128, 128], F16, tag="tr")
                        nc.tensor.transpose(
                            tps[:], p_tiles[tt][:, st * 128:(st + 1) * 128],
                            ident[:])
                        nc.vector.tensor_copy(
                            pt_t[:, st, tt * 128:(tt + 1) * 128], tps[:])

                # --- cT = enc_nat.T @ pT -> [k, t] f16 ---
                ct_t = ctp.tile([128, HT, T], F16, tag="ct")
                for kt in range(HT):
                    cps = psmm.tile([128, T], F32, tag="mm")
                    for st in range(ST):
                        nc.tensor.matmul(
                            cps[:],
                            en_t[:, st, kt * 128:(kt + 1) * 128],
                            pt_t[:, st, :],
                            start=(st == 0), stop=(st == ST - 1))
                    nc.vector.tensor_copy(ct_t[:, kt, :], cps[:])

                # --- out = tanh(cT.T @ WcT + qT.T @ WqT + b) ---
                for tt in range(TT):
                    for hc in range(2):
                        ops = psmm.tile([128, 512], F32, tag="mm")
                        # q-part first: gives tail cT evictions extra slack
                        for ht in range(HT):
                            nc.tensor.matmul(
                                ops[:],
                                qh_t[:, ht, tt * 128:(tt + 1) * 128],
                                wo_t[:, HT + ht, hc * 512:(hc + 1) * 512],
                                start=(ht == 0), stop=False)
                        for kt in range(HT):
                            nc.tensor.matmul(
                                ops[:],
                                ct_t[:, kt, tt * 128:(tt + 1) * 128],
                                wo_t[:, kt, hc * 512:(hc + 1) * 512],
                                start=False, stop=(kt == HT - 1))
                        ost = op.tile([128, 512], F32, tag="os")
                        nc.vector.tensor_add(
                            ost[:], ops[:], bias_t[:, hc * 512:(hc + 1) * 512])
                        nc.scalar.activation(
                            out=ost[:], in_=ost[:],
                            func=mybir.ActivationFunctionType.Tanh)
                        nc.sync.dma_start(
                            out_d[b, tt * 128:(tt + 1) * 128,
                                  hc * 512:(hc + 1) * 512],
                            ost[:])

    nc.compile()
    return nc


def _get_nc(has_bias):
    key = ("nc", has_bias)
    if key not in _CACHE:
        _CACHE[key] = _build(MODE, has_bias)
    return _CACHE[key]


def _split16(x):
    hi = x.astype(np.float16)
    lo = (x - hi.astype(np.float32)).astype(np.float32)
    return hi, lo


def _f8(x, scale):
    return (np.asarray(x, np.float32) * np.float32(scale)).astype(
        ml_dtypes.float8_e4m3)


def kernel(query, encoder_outputs, src_lengths, W_in, W_out, b_out):
    query = np.asarray(query, np.float32)
    enc = np.asarray(encoder_outputs, np.float32)
    W_in = np.asarray(W_in, np.float32)
    W_out = np.asarray(W_out, np.float32)
    b_out = np.asarray(b_out, np.float32)

    # host-side layout prep (transposes + fp16 hi/lo splits)
    qT = np.ascontiguousarray(query.transpose(0, 2, 1))        # [B, H, T]
    qh, ql = _split16(qT)
    encT = np.ascontiguousarray(enc.transpose(1, 2, 0))        # [B, H, S]
    eh, el = _split16(encT)
    en = np.ascontiguousarray(enc.transpose(1, 0, 2)).astype(np.float16)
    whf, wlf = _split16(np.ascontiguousarray(W_in.T))          # [H(h), H(i)]
    wo = np.ascontiguousarray(W_out.T).astype(np.float16)      # [2H, H]
    bias = np.ascontiguousarray(
        np.broadcast_to(b_out[None, :], (128, H)), np.float32)

    common = {"wh": whf, "wo": wo, "bias": bias}
    if MODE == "dr":
        common["wh8"] = _f8(whf.astype(np.float32), SC_WH)
        common["wl8"] = _f8(wlf, SC_WL)
    else:
        common["wl"] = wlf.astype(np.float16)

    in_maps = []
    for c in range(NCORES):
        sl = slice(c * BL, (c + 1) * BL)
        m = {
            "qh": np.ascontiguousarray(qh[sl]),
            "eh": np.ascontiguousarray(eh[sl]),
            "en": np.ascontiguousarray(en[sl]),
            **common,
        }
        if MODE == "dr":
            m["qh8"] = _f8(qh[sl].astype(np.float32), SC_QH)
            m["ql8"] = _f8(ql[sl], SC_QL)
            m["eh8"] = _f8(eh[sl].astype(np.float32), SC_EH)
            m["el8"] = _f8(el[sl], SC_EL)
        else:
            m["ql"] = np.ascontiguousarray(ql[sl]).astype(np.float16)
            m["el"] = np.ascontiguousarray(el[sl]).astype(np.float16)
        in_maps.append(m)

    nc = _get_nc(bool(np.any(b_out)))
    trace = bool(int(os.environ.get("KERNEL_TRACE", "0")))
    res = run_bass_kernel_spmd(nc, in_maps, core_ids=list(range(NCORES)),
                               trace=trace)
    if trace:
        _CACHE["last_exec_time_ns"] = res.exec_time_ns
        _CACHE["last_results"] = res
    out = np.concatenate([r["out"] for r in res.results], axis=0)
    return out


# revision 34
# speedup vs baseline: 1.0121x; 1.0038x over previous
"""Trainium2 Bass kernel for nn_Attention_80779744903968.

Reference computation (B=32, T=512, S=1024, H=1024):
    z      = q @ W_in.T                  [B,T,H]
    scores = z @ enc_b.T                 [B,T,S]   (enc input is [S,B,H])
    p      = softmax(scores, axis=-1)    (the scores==0 -> -inf fill is a
                                          numerical no-op: row maxes are ~120,
                                          exp(0-max) == 0 in fp32)
    c      = p @ enc_b                   [B,T,H]
    out    = tanh([c, q] @ W_out.T + b)  [B,T,H]

Sharding: data-parallel over B across 8 cores (4 batches per core).
W_in / W_out replicated.

Precision strategy (PE matmuls):
  - z and scores need near-fp32 logits: the softmax is near-one-hot
    (logit std ~37) with near-tied rows. Both are computed as an fp16
    hi/lo split: x*y ~= xh*yh (fp16 main pass, fp32 PSUM accumulation)
    + (xh*yl + xl*yh) correction passes.
  - mode "f16x3": corrections in fp16 (absmax ~1.6e-3 vs reference).
  - mode "dr": corrections in fp8(e4m3) with perf_mode=DoubleRow at 2
    contraction-tiles per matmul (absmax ~2.5e-3); correction operands are
    pre-scaled by powers of 2 so both corr products share one PSUM scale.
  - downstream (p, enc, c, q, W_out) runs in plain fp16: p is in [0,1] and
    c/out magnitudes are O(1), so fp16's 2^-11 relative error is plenty.

All transposes (q -> [H,T], enc -> [H,S] per batch) are done on the host so
every device-side DMA is a contiguous natural-layout load; only the softmax
output p is transposed on-device (PE transpose-mode, fp16, 128x128 tiles).
"""
import os
import sys

import numpy as np

sys.path.insert(0, "/opt/trn_rl_repo")

import ml_dtypes  # noqa: E402

import concourse.bass as bass  # noqa: E402
import concourse.tile as tile  # noqa: E402
from concourse import bacc, mybir  # noqa: E402
from concourse.bass_utils import run_bass_kernel_spmd  # noqa: E402
from concourse.masks import make_identity  # noqa: E402

B, T, S, H = 32, 512, 1024, 1024
NCORES = 8
BL = B // NCORES  # batches per core
HT = H // 128     # h/i/k tiles per 1024
TT = T // 128     # t tiles
ST = S // 128     # s tiles
F16 = mybir.dt.float16
F32 = mybir.dt.float32
F8 = mybir.dt.float8e4
DR = mybir.MatmulPerfMode.DoubleRow

MODE = os.environ.get("KERNEL_MODE", "dr")

# power-of-2 scales for fp8 correction operands (products must share scale)
SC_WH, SC_WL, SC_QH, SC_QL = 2.0**4, 2.0**15, 1.0, 2.0**11   # z corr: 2^15
SC_ZH, SC_ZL, SC_EH, SC_EL = 1.0, 2.0**12, 1.0, 2.0**12      # s corr: 2^12

_CACHE = {}


def _build(mode, has_bias):
    nc = bacc.Bacc("TRN2", target_bir_lowering=False, debug=False,
                   num_devices=NCORES)

    def din(name, shape, dt=F16):
        return nc.dram_tensor(name, shape, dt, kind="ExternalInput").ap()

    qh_d = din("qh", [BL, H, T])
    eh_d = din("eh", [BL, H, S])
    en_d = din("en", [BL, S, H])
    wh_d = din("wh", [H, H])
    wo_d = din("wo", [2 * H, H])
    bias_d = din("bias", [128, H], F32)
    if mode == "dr":
        ql8_d = din("ql8", [BL, H, T], F8)
        el8_d = din("el8", [BL, H, S], F8)
        qh8_d = din("qh8", [BL, H, T], F8)
        eh8_d = din("eh8", [BL, H, S], F8)
        wh8_d = din("wh8", [H, H], F8)
        wl8_d = din("wl8", [H, H], F8)
    else:
        wl_d = din("wl", [H, H])
        ql_d = din("ql", [BL, H, T])
        el_d = din("el", [BL, H, S])
    out_d = nc.dram_tensor("out", [BL, T, H], F32, kind="ExternalOutput").ap()

    with tile.TileContext(nc) as tc:
        with (
            tc.tile_pool(name="weights", bufs=1) as wp,
            tc.tile_pool(name="qin", bufs=2) as qp,
            tc.tile_pool(name="ein", bufs=1) as ep,
            tc.tile_pool(name="enin", bufs=1) as enp,
            tc.tile_pool(name="zbuf", bufs=1) as zp,
            tc.tile_pool(name="scratch", bufs=2) as scrp,
            tc.tile_pool(name="scores", bufs=2) as scp,
            tc.tile_pool(name="pbuf", bufs=3) as pp,
            tc.tile_pool(name="ptbuf", bufs=1) as ptp,
            tc.tile_pool(name="ctbuf", bufs=1) as ctp,
            tc.tile_pool(name="ostage", bufs=3) as op,
            tc.tile_pool(name="stats", bufs=8) as stp,
            tc.tile_pool(name="psmm", bufs=4, space="PSUM") as psmm,
            tc.tile_pool(name="psc", bufs=2, space="PSUM") as psc,
            tc.tile_pool(name="pstr", bufs=2, space="PSUM") as pstr,
        ):
            # --- resident weights / constants ---
            # queue order matters at startup: wh then b0's qh go first so the
            # PE can start the z main pass after only 3MB of DMA.
            # chunk wh/qh0 by ht so the first z matmul waits on ~384KB,
            # not the whole 3MB (DMA completion sems are per-transfer)
            wh_t = wp.tile([128, HT, H], F16)
            wh_r = wh_d.rearrange("(ht p) i -> p ht i", p=128)
            qh_first = qp.tile([128, HT, T], F16, tag="qh")
            qh_r = qh_d[0].rearrange("(ht p) t -> p ht t", p=128)
            for ht in range(HT):
                nc.sync.dma_start(wh_t[:, ht, :], wh_r[:, ht, :])
                nc.sync.dma_start(qh_first[:, ht, :], qh_r[:, ht, :])
            if mode == "dr":
                wl8_t = wp.tile([128, HT, H], F8)
                nc.sync.dma_start(
                    wl8_t[:], wl8_d.rearrange("(ht p) i -> p ht i", p=128))
                wh8_t = wp.tile([128, HT, H], F8)
                nc.sync.dma_start(
                    wh8_t[:], wh8_d.rearrange("(ht p) i -> p ht i", p=128))
                qh8_first = qp.tile([128, HT, T], F8, tag="qh8", bufs=1)
                nc.sync.dma_start(
                    qh8_first[:], qh8_d[0].rearrange("(ht p) t -> p ht t", p=128))
                ql8_first = qp.tile([128, HT, T], F8, tag="ql8", bufs=1)
                nc.sync.dma_start(
                    ql8_first[:], ql8_d[0].rearrange("(ht p) t -> p ht t", p=128))
            else:
                wl_t = wp.tile([128, HT, H], F16)
                nc.sync.dma_start(
                    wl_t[:], wl_d.rearrange("(ht p) i -> p ht i", p=128))
            ident = wp.tile([128, 128], F16)
            make_identity(nc, ident[:])
            wo_t = None
            bias_t = None

            for b in range(BL):
                if b == 0:
                    qh_t = qh_first
                else:
                    qh_t = qp.tile([128, HT, T], F16, tag="qh")
                    nc.sync.dma_start(
                        qh_t[:], qh_d[b].rearrange("(ht p) t -> p ht t", p=128))
                if mode == "dr":
                    if b == 0:
                        qh8_t, ql8_t = qh8_first, ql8_first
                    else:
                        qh8_t = qp.tile([128, HT, T], F8, tag="qh8", bufs=1)
                        nc.sync.dma_start(
                            qh8_t[:], qh8_d[b].rearrange("(ht p) t -> p ht t", p=128))
                        ql8_t = qp.tile([128, HT, T], F8, tag="ql8", bufs=1)
                        nc.sync.dma_start(
                            ql8_t[:], ql8_d[b].rearrange("(ht p) t -> p ht t", p=128))
                else:
                    ql_t = qp.tile([128, HT, T], F16, tag="ql")
                    nc.sync.dma_start(
                        ql_t[:], ql_d[b].rearrange("(ht p) t -> p ht t", p=128))
                eh_t = ep.tile([128, HT, S], F16, tag="eh")
                nc.sync.dma_start(
                    eh_t[:], eh_d[b].rearrange("(it p) s -> p it s", p=128))
                if mode == "dr":
                    eh8_t = ep.tile([128, HT, S], F8, tag="eh8")
                    nc.sync.dma_start(
                        eh8_t[:], eh8_d[b].rearrange("(it p) s -> p it s", p=128))
                    el8_t = ep.tile([128, HT, S], F8, tag="el8")
                    nc.sync.dma_start(
                        el8_t[:], el8_d[b].rearrange("(it p) s -> p it s", p=128))
                else:
                    el_t = ep.tile([128, HT, S], F16, tag="el")
                    nc.sync.dma_start(
                        el_t[:], el_d[b].rearrange("(it p) s -> p it s", p=128))
                if wo_t is None:
                    # deferred: b0's scores inputs take queue priority
                    wo_t = wp.tile([128, 2 * HT, H], F16)
                    nc.sync.dma_start(
                        wo_t[:], wo_d.rearrange("(kt p) h -> p kt h", p=128))
                    bias_t = wp.tile([128, H], F32)
                    nc.sync.dma_start(bias_t[:], bias_d)
                en_t = enp.tile([128, ST, H], F16, tag="en")
                nc.sync.dma_start(
                    en_t[:], en_d[b].rearrange("(st p) k -> p st k", p=128))

                # --- zT = W_inT.T @ qT (hi/lo) -> zh (f16) + lo forms ---
                zh_t = zp.tile([128, HT, T], F16, tag="zh")
                if mode == "dr":
                    zh8_t = zp.tile([128, HT, T], F8, tag="zh8")
                    zl8_t = zp.tile([128, HT, T], F8, tag="zl8")
                else:
                    zl_t = zp.tile([128, HT, T], F16, tag="zl")

                def z_evict(it, zps, zcorr=None):
                    if mode == "dr":
                        comb = scrp.tile([128, T], F32, tag="comb")
                        nc.vector.tensor_copy(comb[:], zps[:])
                        nc.vector.scalar_tensor_tensor(
                            out=comb[:], in0=zcorr[:],
                            scalar=1.0 / (SC_WH * SC_QL), in1=comb[:],
                            op0=mybir.AluOpType.mult, op1=mybir.AluOpType.add)
                        nc.vector.tensor_copy(zh_t[:, it, :], comb[:])
                        zl_tmp = scrp.tile([128, T], F16, tag="zltmp")
                        nc.vector.tensor_sub(zl_tmp[:], comb[:], zh_t[:, it, :])
                        nc.vector.tensor_scalar_mul(
                            zl8_t[:, it, :], zl_tmp[:], SC_ZL)
                        nc.vector.tensor_copy(zh8_t[:, it, :], zh_t[:, it, :])
                    else:
                        nc.vector.tensor_copy(zh_t[:, it, :], zps[:])
                        nc.vector.tensor_sub(zl_t[:, it, :], zps[:],
                                             zh_t[:, it, :])

                def z_corr_mms(zcorr, it, first, last):
                    # fp8 DoubleRow corrections: (wl8*qh8 + wh8*ql8) * 2^-15
                    j = 0
                    n = HT  # 2 passes x HT/2 pair-matmuls
                    for lhs, rhs in ((wl8_t, qh8_t), (wh8_t, ql8_t)):
                        for k in range(HT // 2):
                            nc.tensor.matmul(
                                zcorr[:],
                                lhs[:, 2 * k:2 * k + 2, it * 128:(it + 1) * 128],
                                rhs[:, 2 * k:2 * k + 2, :],
                                start=(first and j == 0),
                                stop=(last and j == n - 1),
                                perf_mode=DR)
                            j += 1

                if b == 0 and mode != "dr":
                    # pass-major over quads of i-tiles: the first 32 matmuls
                    # need only wh+qh (3MB of DMA), so the PE starts early.
                    for quad in range(HT // 4):
                        its = range(quad * 4, quad * 4 + 4)
                        zpss = {it: psmm.tile([128, T], F32, tag="mm", name=f"zps{it}")
                                for it in its}
                        if mode == "dr":
                            zcorrs = {it: psc.tile([128, T], F32, tag="mmc", name=f"zcorr{it}")
                                      for it in its}
                            for it in its:
                                for ht in range(HT):
                                    nc.tensor.matmul(
                                        zpss[it][:],
                                        wh_t[:, ht, it * 128:(it + 1) * 128],
                                        qh_t[:, ht, :],
                                        start=(ht == 0), stop=(ht == HT - 1))
                            for it in its:
                                z_corr_mms(zcorrs[it], it, True, True)
                            for it in its:
                                z_evict(it, zpss[it], zcorrs[it])
                        else:
                            passes = ((wh_t, qh_t), (wl_t, qh_t), (wh_t, ql_t))
                            for pi, (lhs, rhs) in enumerate(passes):
                                for it in its:
                                    for ht in range(HT):
                                        nc.tensor.matmul(
                                            zpss[it][:],
                                            lhs[:, ht, it * 128:(it + 1) * 128],
                                            rhs[:, ht, :],
                                            start=(pi == 0 and ht == 0),
                                            stop=(pi == 2 and ht == HT - 1))
                            for it in its:
                                z_evict(it, zpss[it])
                else:
                    for it in range(HT):
                        zps = psmm.tile([128, T], F32, tag="mm")
                        if mode == "dr":
                            for ht in range(HT):
                                nc.tensor.matmul(
                                    zps[:],
                                    wh_t[:, ht, it * 128:(it + 1) * 128],
                                    qh_t[:, ht, :],
                                    start=(ht == 0), stop=(ht == HT - 1))
                            zcorr = psc.tile([128, T], F32, tag="mmc")
                            z_corr_mms(zcorr, it, True, True)
                            z_evict(it, zps, zcorr)
                        else:
                            j = 0
                            for lhs, rhs in ((wh_t, qh_t), (wh_t, ql_t),
                                             (wl_t, qh_t)):
                                for ht in range(HT):
                                    nc.tensor.matmul(
                                        zps[:],
                                        lhs[:, ht, it * 128:(it + 1) * 128],
                                        rhs[:, ht, :],
                                        start=(j == 0), stop=(j == 3 * HT - 1))
                                    j += 1
                            z_evict(it, zps)

                # --- scores + softmax -> p (f16, normalized) ---
                p_tiles = []
                for tt in range(TT):
                    sc_t = scp.tile([128, S], F32, tag="sc")
                    for sc in range(2):
                        sps = psmm.tile([128, 512], F32, tag="mm")
                        for it in range(HT):
                            nc.tensor.matmul(
                                sps[:],
                                zh_t[:, it, tt * 128:(tt + 1) * 128],
                                eh_t[:, it, sc * 512:(sc + 1) * 512],
                                start=(it == 0),
                                stop=(mode == "dr" and it == HT - 1))
                        if mode == "dr":
                            scorr = psc.tile([128, 512], F32, tag="mmc")
                            j = 0
                            for lhs, rhs in ((zl8_t, eh8_t), (zh8_t, el8_t)):
                                for k in range(HT // 2):
                                    nc.tensor.matmul(
                                        scorr[:],
                                        lhs[:, 2 * k:2 * k + 2,
                                            tt * 128:(tt + 1) * 128],
                                        rhs[:, 2 * k:2 * k + 2,
                                            sc * 512:(sc + 1) * 512],
                                        start=(j == 0), stop=(j == HT - 1),
                                        perf_mode=DR)
                                    j += 1
                            chunk = sc_t[:, sc * 512:(sc + 1) * 512]
                            nc.vector.tensor_copy(chunk, sps[:])
                            nc.vector.scalar_tensor_tensor(
                                out=chunk, in0=scorr[:],
                                scalar=1.0 / (SC_ZL * SC_EH), in1=chunk,
                                op0=mybir.AluOpType.mult,
                                op1=mybir.AluOpType.add)
                        else:
                            j = 0
                            for lhs, rhs in ((zh_t, el_t), (zl_t, eh_t)):
                                for it in range(HT):
                                    nc.tensor.matmul(
                                        sps[:],
                                        lhs[:, it, tt * 128:(tt + 1) * 128],
                                        rhs[:, it, sc * 512:(sc + 1) * 512],
                                        start=False, stop=(j == 2 * HT - 1))
                                    j += 1
                            nc.vector.tensor_copy(
                                sc_t[:, sc * 512:(sc + 1) * 512], sps[:])
                    # softmax over free dim (s)
                    negmax = stp.tile([128, 1], F32, tag="nm")
                    nc.vector.reduce_max(out=negmax[:], in_=sc_t[:],
                                         axis=mybir.AxisListType.X, negate=True)
                    p_t = pp.tile([128, S], F16, tag="p")
                    nc.scalar.activation(
                        out=p_t[:], in_=sc_t[:],
                        func=mybir.ActivationFunctionType.Exp,
                        bias=negmax[:], scale=1.0)
                    ssum = stp.tile([128, 1], F32, tag="ss")
                    nc.vector.reduce_sum(out=ssum[:], in_=p_t[:],
                                         axis=mybir.AxisListType.X)
                    rsum = stp.tile([128, 1], F32, tag="rs")
                    nc.vector.reciprocal(rsum[:], ssum[:])
                    nc.vector.tensor_scalar_mul(p_t[:], p_t[:], rsum[:])
                    p_tiles.append(p_t)

                # --- transpose p -> pT [s, t] (PE transpose 128x128) ---
                pt_t = ptp.tile([128, ST, T], F16, tag="pt")
                for tt in range(TT):
                    for st in range(ST):
                        tps = pstr.tile([128, 128], F16, tag="tr")
                        nc.tensor.transpose(
                            tps[:], p_tiles[tt][:, st * 128:(st + 1) * 128],
                            ident[:])
                        nc.vector.tensor_copy(
                            pt_t[:, st, tt * 128:(tt + 1) * 128], tps[:])

                # --- cT = enc_nat.T @ pT -> [k, t] f16 ---
                ct_t = ctp.tile([128, HT, T], F16, tag="ct")
                for kt in range(HT):
                    cps = psmm.tile([128, T], F32, tag="mm")
                    for st in range(ST):
                        nc.tensor.matmul(
                            cps[:],
                            en_t[:, st, kt * 128:(kt + 1) * 128],
                            pt_t[:, st, :],
                            start=(st == 0), stop=(st == ST - 1))
                    nc.vector.tensor_copy(ct_t[:, kt, :], cps[:])

                # --- out = tanh(cT.T @ WcT + qT.T @ WqT + b) ---
                for tt in range(TT):
                    for hc in range(2):
                        ops = psmm.tile([128, 512], F32, tag="mm")
                        # q-part first: gives tail cT evictions extra slack
                        for ht in range(HT):
                            nc.tensor.matmul(
                                ops[:],
                                qh_t[:, ht, tt * 128:(tt + 1) * 128],
                                wo_t[:, HT + ht, hc * 512:(hc + 1) * 512],
                                start=(ht == 0), stop=False)
                        for kt in range(HT):
                            nc.tensor.matmul(
                                ops[:],
                                ct_t[:, kt, tt * 128:(tt + 1) * 128],
                                wo_t[:, kt, hc * 512:(hc + 1) * 512],
                                start=False, stop=(kt == HT - 1))
                        ost = op.tile([128, 512], F32, tag="os")
                        nc.vector.tensor_add(
                            ost[:], ops[:], bias_t[:, hc * 512:(hc + 1) * 512])
                        nc.scalar.activation(
                            out=ost[:], in_=ost[:],
                            func=mybir.ActivationFunctionType.Tanh)
                        nc.sync.dma_start(
                            out_d[b, tt * 128:(tt + 1) * 128,
                                  hc * 512:(hc + 1) * 512],
                            ost[:])

    nc.compile()
    return nc


def _get_nc(has_bias):
    key = ("nc", has_bias)
    if key not in _CACHE:
        _CACHE[key] = _build(MODE, has_bias)
    return _CACHE[key]


def _split16(x):
    hi = x.astype(np.float16)
    lo = (x - hi.astype(np.float32)).astype(np.float32)
    return hi, lo


def _f8(x, scale):
    return (np.asarray(x, np.float32) * np.float32(scale)).astype(
        ml_dtypes.float8_e4m3)


def kernel(query, encoder_outputs, src_lengths, W_in, W_out, b_out):
    query = np.asarray(query, np.float32)
    enc = np.asarray(encoder_outputs, np.float32)
    W_in = np.asarray(W_in, np.float32)
    W_out = np.asarray(W_out, np.float32)
    b_out = np.asarray(b_out, np.float32)

    # host-side layout prep (transposes + fp16 hi/lo splits)
    qT = np.ascontiguousarray(query.transpose(0, 2, 1))        # [B, H, T]
    qh, ql = _split16(qT)
    encT = np.ascontiguousarray(enc.transpose(1, 2, 0))        # [B, H, S]
    eh, el = _split16(encT)
    en = np.ascontiguousarray(enc.transpose(1, 0, 2)).astype(np.float16)
    whf, wlf = _split16(np.ascontiguousarray(W_in.T))          # [H(h), H(i)]
    wo = np.ascontiguousarray(W_out.T).astype(np.float16)      # [2H, H]
    bias = np.ascontiguousarray(
        np.broadcast_to(b_out[None, :], (128, H)), np.float32)

    common = {"wh": whf, "wo": wo, "bias": bias}
    if MODE == "dr":
        common["wh8"] = _f8(whf.astype(np.float32), SC_WH)
        common["wl8"] = _f8(wlf, SC_WL)
    else:
        common["wl"] = wlf.astype(np.float16)

    in_maps = []
    for c in range(NCORES):
        sl = slice(c * BL, (c + 1) * BL)
        m = {
            "qh": np.ascontiguousarray(qh[sl]),
            "eh": np.ascontiguousarray(eh[sl]),
            "en": np.ascontiguousarray(en[sl]),
            **common,
        }
        if MODE == "dr":
            m["qh8"] = _f8(qh[sl].astype(np.float32), SC_QH)
            m["ql8"] = _f8(ql[sl], SC_QL)
            m["eh8"] = _f8(eh[sl].astype(np.float32), SC_EH)
            m["el8"] = _f8(el[sl], SC_EL)
        else:
            m["ql"] = np.ascontiguousarray(ql[sl]).astype(np.float16)
            m["el"] = np.ascontiguousarray(el[sl]).astype(np.float16)
        in_maps.append(m)

    nc = _get_nc(bool(np.any(b_out)))
    trace = bool(int(os.environ.get("KERNEL_TRACE", "0")))
    res = run_bass_kernel_spmd(nc, in_maps, core_ids=list(range(NCORES)),
                               trace=trace)
    if trace:
        _CACHE["last_exec_time_ns"] = res.exec_time_ns
        _CACHE["last_results"] = res
    out = np.concatenate([r["out"] for r in res.results], axis=0)
    return out
